# revision 5
# baseline (speedup 1.0000x reference)
"""Trainium2 Bass kernel for the ContractiveREN problem.

Strategy
--------
Data parallel over the batch: each of the 8 NeuronCores gets a 2048-row
shard of ``u_in``; all (small) parameter matrices are folded on the host
into four 128x128 fp16 matmul weights plus two per-partition fp32 bias
vectors.

Math
----
The reference computes (per batch row u, with x0 the initial state):
    w_i   = tanh((xc_i + ud_i + sum_{j<i} D11_ij w_j) / Lam_i)   (i = 0..127)
    y     = u @ Gu^T + w @ Gw^T + c0
where everything except the w-recurrence is affine in (u, w) and folds into
    Lhat = D11 / Lam[:,None],  xcl = xc/Lam,  UD = (D12/Lam) @ u^T
    Gu   = C2 @ inv(E) @ B2 + D22,  Gw = C2 @ inv(E) @ B1 + D21
    c0   = C2 @ inv(E) @ F @ x0
The strictly-lower-triangular recurrence is solved by fixed-point
iteration  W <- tanh(Lhat @ W + UD + xcl), which contracts the error by
~3.2x per sweep; 4 sweeps give rel err ~1e-3 against the fp32 reference
(the harness gate is 2e-2).

Implementation notes (all fp16 data / fp32 PSUM accumulation):
  * u is cast to fp16 on the host and loaded feature-major via the DMA
    XBAR transpose (dma_start_transpose) - no PE transposes at all.
  * Each 512-batch chunk owns one live PSUM bank holding
    UD + Lhat@W_k: pass k accumulates Lhat@(W_k - W_{k-1}) via matmul
    (the W-delta is a cheap all-fp16 DVE subtract), and every tanh
    applies xcl as the ACT bias, so no UDb tensor and no fp32 adds.
  * Output is computed feature-major (Gu@Ut + Gw@W accumulated in PSUM,
    + c0 as a DVE per-partition scalar add) and stored to DRAM
    feature-major as fp16; the host transposes/upcasts per core.
"""

import numpy as np

import concourse.bass as bass
import concourse.mybir as mybir
import concourse.tile as tile
from concourse import bacc
from concourse.bass_utils import run_bass_kernel_spmd

B = 16384
N_CORES = 8
BC = B // N_CORES  # 2048 batch rows per core
DIM_IN = 128
DIM_OUT = 128
DIM_X = 512
DIM_NL = 128
DIM_H = 2 * DIM_X + DIM_NL
EPS = 1e-3
ALPHA = 1.0
NCH = 4            # 512-column batch chunks (one PSUM bank each)
CW = BC // NCH     # 512
M_PASSES = 3       # delta-Jacobi passes after the seeded first sweep
F16 = mybir.dt.float16
F32 = mybir.dt.float32
TANH = mybir.ActivationFunctionType.Tanh

_BUILT = {}


def _build_nc():
    nc = bacc.Bacc("TRN2", target_bir_lowering=False, debug=False)
    u = nc.dram_tensor("u", [BC, DIM_IN], F16, kind="ExternalInput").ap()
    cst = nc.dram_tensor("cst", [128, 516], F16, kind="ExternalInput").ap()
    y = nc.dram_tensor("y", [DIM_OUT, BC], F16, kind="ExternalOutput").ap()

    u_g = u.rearrange("(g r) f -> g r f", g=NCH)

    PW = 2 * CW  # 1024: two chunks (= two PSUM banks) per "pair"
    with tile.TileContext(nc) as tc:
        with (
            tc.tile_pool(name="const", bufs=1) as cpool,
            tc.tile_pool(name="big", bufs=1) as bpool,
            tc.tile_pool(name="w", bufs=2) as wpool,
            tc.tile_pool(name="d", bufs=2) as dpool,
            tc.tile_pool(name="yst", bufs=1) as ypool,
            tc.tile_pool(name="ps", bufs=1, space="PSUM") as pspool,
            tc.tile_pool(name="po", bufs=1, space="PSUM") as popool,
        ):
            # -- tanh table warm-up on ACT while input DMAs are in flight --
            tiny = cpool.tile([128, 1], F32, tag="tiny")
            nc.gpsimd.memset(tiny[:], 0.0)
            tiny2 = cpool.tile([128, 1], F32, tag="tiny2")

            # -- input DMAs: constants first on the ACT HWDGE queue, then
            #    u feature-major via XBAR transposes split across the two
            #    HWDGE queues (SP gets g=0,2; ACT gets g=1,3) --
            cst_t = cpool.tile([128, 516], F16, tag="cst")
            nc.scalar.dma_start(cst_t[:], cst)
            nc.scalar.activation(tiny2[:], tiny[:], TANH)
            ut = bpool.tile([128, BC], F16, tag="ut")
            for g in range(NCH):
                eng = nc.sync if g % 2 == 0 else nc.scalar
                eng.dma_start_transpose(ut[:, g * CW:(g + 1) * CW], u_g[g])

            ltr = cst_t[:, 0:128]      # Lhat^T
            d12lt = cst_t[:, 128:256]  # (D12/Lam)^T
            gut = cst_t[:, 256:384]    # Gu^T
            gwt = cst_t[:, 384:512]    # Gw^T
            xcl = cst_t[:, 512:514].bitcast(F32)  # xc/Lam      [128,1] f32
            c0 = cst_t[:, 514:516].bitcast(F32)   # C2 Einv F x0 [128,1] f32

            ps = [
                pspool.tile([128, PW], F32, tag=f"ps{p}", name=f"ps{p}")
                for p in range(2)
            ]
            po = [
                popool.tile([128, PW], F32, tag=f"po{p}", name=f"po{p}")
                for p in range(2)
            ]

            def halves(t):
                # chunk n -> (pair tile, in-pair column slice)
                return [(t[n // 2], slice((n % 2) * CW, (n % 2 + 1) * CW))
                        for n in range(NCH)]

            ps_h, po_h = halves(ps), halves(po)

            # -- seed: ps = UD (per chunk-half); Gu@Ut early-fills po --
            for n in range(NCH):
                sl = slice(n * CW, (n + 1) * CW)
                t, hs = ps_h[n]
                nc.tensor.matmul(t[:, hs], d12lt, ut[:, sl],
                                 start=True, stop=False)
            for n in range(NCH):
                sl = slice(n * CW, (n + 1) * CW)
                t, hs = po_h[n]
                nc.tensor.matmul(t[:, hs], gut, ut[:, sl],
                                 start=True, stop=False)

            # -- W1 = tanh(ps + xcl), one ACT call per pair --
            w_cur = [None, None]
            for p in range(2):
                wt = wpool.tile([128, PW], F16, tag=f"w{p}", name=f"w{p}")
                nc.scalar.activation(wt[:], ps[p][:], TANH, bias=xcl)
                w_cur[p] = wt

            # -- delta-Jacobi passes: ps += Lhat @ (W_k - W_{k-1}) --
            w_prev = [None, None]
            for m in range(M_PASSES):
                last = m == M_PASSES - 1
                for p in range(2):
                    if m == 0:
                        dl = w_cur[p]  # W1 - 0
                    else:
                        dl = dpool.tile([128, PW], F16, tag=f"d{p}",
                                        name=f"d{p}")
                        nc.vector.tensor_sub(dl[:], w_cur[p][:], w_prev[p][:])
                    for h in range(2):
                        hs = slice(h * CW, (h + 1) * CW)
                        nc.tensor.matmul(ps[p][:, hs], ltr, dl[:, hs],
                                         start=False, stop=last)
                for p in range(2):
                    w_prev[p] = w_cur[p]
                    wt = wpool.tile([128, PW], F16, tag=f"w{p}", name=f"w{p}")
                    nc.scalar.activation(wt[:], ps[p][:], TANH, bias=xcl)
                    w_cur[p] = wt

            # -- output: po (= Gu@Ut) += Gw@W; yt = po + c0; store.
            #    c0-adds split DVE/Pool, store DMAs split SP/ACT queues --
            for n in range(NCH):
                t, hs = po_h[n]
                nc.tensor.matmul(t[:, hs], gwt, w_cur[n // 2][:, hs],
                                 start=False, stop=True)
            for p in range(2):
                sl = slice(p * PW, (p + 1) * PW)
                yts = ypool.tile([128, PW], F16, tag=f"yt{p}", name=f"yt{p}")
                nc.vector.tensor_scalar_add(yts[:], po[p][:], c0)
                deng = nc.sync if p == 0 else nc.scalar
                deng.dma_start(y[:, sl], yts[:])
    nc.compile()
    return nc


def _derive_cst(X, Y, B2, C2, D21, D22, D12, x0):
    """Fold the contractive parameterization into kernel constants."""
    f = np.float32
    X = np.ascontiguousarray(X, f)
    H = (X.T @ X + EPS * np.eye(DIM_H, dtype=f)).astype(f)
    H11 = H[:DIM_X, :DIM_X]
    H21 = H[DIM_X:DIM_X + DIM_NL, :DIM_X]
    H22 = H[DIM_X:DIM_X + DIM_NL, DIM_X:DIM_X + DIM_NL]
    H31 = H[DIM_X + DIM_NL:, :DIM_X]
    H32 = H[DIM_X + DIM_NL:, DIM_X:DIM_X + DIM_NL]
    H33 = H[DIM_X + DIM_NL:, DIM_X + DIM_NL:]
    F = H31
    B1 = H32
    E = (0.5 * (H11 + ALPHA * H33 + Y - Y.T)).astype(f)
    Lam = (0.5 * np.diagonal(H22)).astype(f)
    D11 = (-np.tril(H22, k=-1)).astype(f)
    C1 = -H21

    Einv = np.linalg.inv(E).astype(f)
    x0v = np.asarray(x0, f)[0, 0, :]
    xc = (C1 @ x0v).astype(f)
    fx = (F @ x0v).astype(f)

    Lhat = (D11 / Lam[:, None]).astype(f)
    D12L = (np.asarray(D12, f) / Lam[:, None]).astype(f)
    CE = (np.asarray(C2, f) @ Einv).astype(f)
    Gu = (CE @ B2 + D22).astype(f)
    Gw = (CE @ B1 + D21).astype(f)
    xclam = (xc / Lam).astype(f)
    c0 = (CE @ fx).astype(f)

    cst = np.zeros((128, 516), np.float16)
    cst[:, 0:128] = Lhat.T.astype(np.float16)
    cst[:, 128:256] = D12L.T.astype(np.float16)
    cst[:, 256:384] = Gu.T.astype(np.float16)
    cst[:, 384:512] = Gw.T.astype(np.float16)
    cst[:, 512:514] = xclam.reshape(128, 1).view(np.float16)
    cst[:, 514:516] = c0.reshape(128, 1).view(np.float16)
    return cst


def _make_in_maps(u_in, X, Y, B2, C2, D21, D22, D12, x0):
    cst = _derive_cst(X, Y, B2, C2, D21, D22, D12, x0)
    u16 = np.ascontiguousarray(
        np.asarray(u_in, np.float32).reshape(B, DIM_IN).astype(np.float16)
    )
    return [
        {"u": u16[i * BC:(i + 1) * BC], "cst": cst}
        for i in range(N_CORES)
    ]


def kernel(u_in, X, Y, B2, C2, D21, D22, D12, x0):
    if "nc" not in _BUILT:
        _BUILT["nc"] = _build_nc()
    nc = _BUILT["nc"]
    in_maps = _make_in_maps(u_in, X, Y, B2, C2, D21, D22, D12, x0)
    res = run_bass_kernel_spmd(nc, in_maps, core_ids=list(range(N_CORES)))
    out = np.concatenate(
        [res.results[i]["y"].astype(np.float32).T for i in range(N_CORES)],
        axis=0,
    )
    return out.reshape(B, 1, DIM_OUT)


# revision 6
# speedup vs baseline: 1.0781x; 1.0781x over previous
"""Trainium2 Bass kernel for the ContractiveREN problem.

Strategy
--------
Data parallel over the batch: each of the 8 NeuronCores gets a 2048-row
shard of ``u_in``; all (small) parameter matrices are folded on the host
into four 128x128 fp16 matmul weights plus two per-partition fp32 bias
vectors.

Math
----
The reference computes (per batch row u, with x0 the initial state):
    w_i   = tanh((xc_i + ud_i + sum_{j<i} D11_ij w_j) / Lam_i)   (i = 0..127)
    y     = u @ Gu^T + w @ Gw^T + c0
where everything except the w-recurrence is affine in (u, w) and folds into
    Lhat = D11 / Lam[:,None],  xcl = xc/Lam,  UD = (D12/Lam) @ u^T
    Gu   = C2 @ inv(E) @ B2 + D22,  Gw = C2 @ inv(E) @ B1 + D21
    c0   = C2 @ inv(E) @ F @ x0
The strictly-lower-triangular recurrence is solved by fixed-point
iteration  W <- tanh(Lhat @ W + UD + xcl), which contracts the error by
~3.2x per sweep; 4 sweeps give rel err ~1e-3 against the fp32 reference
(the harness gate is 2e-2).

Implementation notes (all fp16 data / fp32 PSUM accumulation):
  * u is cast to fp16 on the host and loaded feature-major via the DMA
    XBAR transpose (dma_start_transpose) - no PE transposes at all.
  * Each 512-batch chunk owns one live PSUM bank holding
    UD + Lhat@W_k: pass k accumulates Lhat@(W_k - W_{k-1}) via matmul
    (the W-delta is a cheap all-fp16 DVE subtract), and every tanh
    applies xcl as the ACT bias, so no UDb tensor and no fp32 adds.
  * Output is computed feature-major (Gu@Ut + Gw@W accumulated in PSUM,
    + c0 as a DVE per-partition scalar add) and stored to DRAM
    feature-major as fp16; the host transposes/upcasts per core.
"""

import numpy as np

import concourse.bass as bass
import concourse.mybir as mybir
import concourse.tile as tile
from concourse import bacc
from concourse.bass_utils import run_bass_kernel_spmd

B = 16384
N_CORES = 8
BC = B // N_CORES  # 2048 batch rows per core
DIM_IN = 128
DIM_OUT = 128
DIM_X = 512
DIM_NL = 128
DIM_H = 2 * DIM_X + DIM_NL
EPS = 1e-3
ALPHA = 1.0
NCH = 4            # 512-column batch chunks (one PSUM bank each)
CW = BC // NCH     # 512
M_PASSES = 3       # delta-Jacobi passes after the seeded first sweep
F16 = mybir.dt.float16
F32 = mybir.dt.float32
TANH = mybir.ActivationFunctionType.Tanh

_BUILT = {}


def _build_nc():
    nc = bacc.Bacc("TRN2", target_bir_lowering=False, debug=False)
    u = nc.dram_tensor("u", [BC, DIM_IN], F16, kind="ExternalInput").ap()
    cst = nc.dram_tensor("cst", [128, 516], F16, kind="ExternalInput").ap()
    y = nc.dram_tensor("y", [DIM_OUT, BC], F16, kind="ExternalOutput").ap()

    u_g = u.rearrange("(g r) f -> g r f", g=NCH)

    IDENT = mybir.ActivationFunctionType.Identity
    u_h = u.rearrange("(g r) f -> g r f", g=2)  # two 1024-row load halves
    with tile.TileContext(nc) as tc:
        with (
            tc.tile_pool(name="const", bufs=1) as cpool,
            tc.tile_pool(name="big", bufs=1) as bpool,
            tc.tile_pool(name="w", bufs=2) as wpool,
            tc.tile_pool(name="d", bufs=2) as dpool,
            tc.tile_pool(name="yst", bufs=1) as ypool,
            tc.tile_pool(name="ps", bufs=1, space="PSUM") as pspool,
            tc.tile_pool(name="po", bufs=1, space="PSUM") as popool,
        ):
            # -- tanh table warm-up on ACT while input DMAs are in flight --
            tiny = cpool.tile([128, 1], F32, tag="tiny")
            nc.gpsimd.memset(tiny[:], 0.0)
            tiny2 = cpool.tile([128, 1], F32, tag="tiny2")

            # -- input DMAs, all on the ACT HWDGE queue (the SP ring's
            #    first DMA picks up a spurious wait on the ACT ring's
            #    first completion, so keep SP for stores only): constants,
            #    then u feature-major via two 1024-row XBAR transposes --
            cst_t = cpool.tile([128, 516], F16, tag="cst")
            nc.scalar.dma_start(cst_t[:], cst)
            nc.scalar.activation(tiny2[:], tiny[:], TANH)
            ut = bpool.tile([128, BC], F16, tag="ut")
            for g in range(2):
                nc.scalar.dma_start_transpose(
                    ut[:, g * 2 * CW:(g + 1) * 2 * CW], u_h[g]
                )

            ltr = cst_t[:, 0:128]      # Lhat^T
            d12lt = cst_t[:, 128:256]  # (D12/Lam)^T
            gut = cst_t[:, 256:384]    # Gu^T
            gwt = cst_t[:, 384:512]    # Gw^T
            xcl = cst_t[:, 512:514].bitcast(F32)  # xc/Lam      [128,1] f32
            c0 = cst_t[:, 514:516].bitcast(F32)   # C2 Einv F x0 [128,1] f32

            ps = [
                pspool.tile([128, CW], F32, tag=f"ps{n}", name=f"ps{n}")
                for n in range(NCH)
            ]
            po = [
                popool.tile([128, CW], F32, tag=f"po{n}", name=f"po{n}")
                for n in range(NCH)
            ]

            # -- seed: ps = UD; Gu@Ut early-fills po while ACT runs tanh --
            for n in range(NCH):
                sl = slice(n * CW, (n + 1) * CW)
                nc.tensor.matmul(ps[n][:], d12lt, ut[:, sl],
                                 start=True, stop=False)
            for n in range(NCH):
                sl = slice(n * CW, (n + 1) * CW)
                nc.tensor.matmul(po[n][:], gut, ut[:, sl],
                                 start=True, stop=False)

            # -- W1 = tanh(ps + xcl) --
            w_cur = [None] * NCH
            for n in range(NCH):
                wt = wpool.tile([128, CW], F16, tag=f"w{n}", name=f"w{n}")
                nc.scalar.activation(wt[:], ps[n][:], TANH, bias=xcl)
                w_cur[n] = wt

            # -- delta-Jacobi passes: ps += Lhat @ (W_k - W_{k-1}) --
            w_prev = [None] * NCH
            for m in range(M_PASSES):
                last = m == M_PASSES - 1
                for n in range(NCH):
                    if m == 0:
                        dl = w_cur[n]  # W1 - 0
                    else:
                        dl = dpool.tile([128, CW], F16, tag=f"d{n}",
                                        name=f"d{n}")
                        nc.vector.tensor_sub(dl[:], w_cur[n][:], w_prev[n][:])
                    nc.tensor.matmul(ps[n][:], ltr, dl[:],
                                     start=False, stop=last)
                for n in range(NCH):
                    w_prev[n] = w_cur[n]
                    wt = wpool.tile([128, CW], F16, tag=f"w{n}", name=f"w{n}")
                    nc.scalar.activation(wt[:], ps[n][:], TANH, bias=xcl)
                    w_cur[n] = wt

            # -- output: po (= Gu@Ut) += Gw@W; yt = po + c0; store.
            #    c0-adds alternate DVE / ACT-Identity (same act table as
            #    tanh, no reload); store DMAs alternate the SP/ACT queues --
            for n in range(NCH):
                nc.tensor.matmul(po[n][:], gwt, w_cur[n][:],
                                 start=False, stop=True)
            for n in range(NCH):
                sl = slice(n * CW, (n + 1) * CW)
                yts = ypool.tile([128, CW], F16, tag=f"yt{n}", name=f"yt{n}")
                if n % 2 == 0:
                    nc.vector.tensor_scalar_add(yts[:], po[n][:], c0)
                else:
                    nc.scalar.activation(yts[:], po[n][:], IDENT, bias=c0)
                deng = nc.sync if n < 2 else nc.scalar
                deng.dma_start(y[:, sl], yts[:])
    nc.compile()
    return nc


def _derive_cst(X, Y, B2, C2, D21, D22, D12, x0):
    """Fold the contractive parameterization into kernel constants."""
    f = np.float32
    X = np.ascontiguousarray(X, f)
    H = (X.T @ X + EPS * np.eye(DIM_H, dtype=f)).astype(f)
    H11 = H[:DIM_X, :DIM_X]
    H21 = H[DIM_X:DIM_X + DIM_NL, :DIM_X]
    H22 = H[DIM_X:DIM_X + DIM_NL, DIM_X:DIM_X + DIM_NL]
    H31 = H[DIM_X + DIM_NL:, :DIM_X]
    H32 = H[DIM_X + DIM_NL:, DIM_X:DIM_X + DIM_NL]
    H33 = H[DIM_X + DIM_NL:, DIM_X + DIM_NL:]
    F = H31
    B1 = H32
    E = (0.5 * (H11 + ALPHA * H33 + Y - Y.T)).astype(f)
    Lam = (0.5 * np.diagonal(H22)).astype(f)
    D11 = (-np.tril(H22, k=-1)).astype(f)
    C1 = -H21

    Einv = np.linalg.inv(E).astype(f)
    x0v = np.asarray(x0, f)[0, 0, :]
    xc = (C1 @ x0v).astype(f)
    fx = (F @ x0v).astype(f)

    Lhat = (D11 / Lam[:, None]).astype(f)
    D12L = (np.asarray(D12, f) / Lam[:, None]).astype(f)
    CE = (np.asarray(C2, f) @ Einv).astype(f)
    Gu = (CE @ B2 + D22).astype(f)
    Gw = (CE @ B1 + D21).astype(f)
    xclam = (xc / Lam).astype(f)
    c0 = (CE @ fx).astype(f)

    cst = np.zeros((128, 516), np.float16)
    cst[:, 0:128] = Lhat.T.astype(np.float16)
    cst[:, 128:256] = D12L.T.astype(np.float16)
    cst[:, 256:384] = Gu.T.astype(np.float16)
    cst[:, 384:512] = Gw.T.astype(np.float16)
    cst[:, 512:514] = xclam.reshape(128, 1).view(np.float16)
    cst[:, 514:516] = c0.reshape(128, 1).view(np.float16)
    return cst


def _make_in_maps(u_in, X, Y, B2, C2, D21, D22, D12, x0):
    cst = _derive_cst(X, Y, B2, C2, D21, D22, D12, x0)
    u16 = np.ascontiguousarray(
        np.asarray(u_in, np.float32).reshape(B, DIM_IN).astype(np.float16)
    )
    return [
        {"u": u16[i * BC:(i + 1) * BC], "cst": cst}
        for i in range(N_CORES)
    ]


def kernel(u_in, X, Y, B2, C2, D21, D22, D12, x0):
    if "nc" not in _BUILT:
        _BUILT["nc"] = _build_nc()
    nc = _BUILT["nc"]
    in_maps = _make_in_maps(u_in, X, Y, B2, C2, D21, D22, D12, x0)
    res = run_bass_kernel_spmd(nc, in_maps, core_ids=list(range(N_CORES)))
    out = np.concatenate(
        [res.results[i]["y"].astype(np.float32).T for i in range(N_CORES)],
        axis=0,
    )
    return out.reshape(B, 1, DIM_OUT)
